# revision 4
# baseline (speedup 1.0000x reference)
"""GAT node encoder (3 GATConv+BN layers) on 8 trn2 NeuronCores — v2.

Sharding: nodes partitioned across cores by global degree-sorted round-robin
dealing (dst-sharded message passing). Per layer, per core:
  1. fp16 matmul of this core's node shard: [h | s | d] = y @ [W | W@a_src | W@a_dst]
  2. Two AllGathers (overlapped with the matmul phase) of the fp16 [h | s]
     node table into Shared DRAM (no bounce copy).
  3. Per dst-tile (128 nodes, ELL layout, self-loops excluded): one indirect-DMA
     row gather per ELL slot, one-pass segment softmax over incoming edges with
     the self-loop term taken from the local SBUF hs tile, fp16 weighted
     accumulation, head mean via PSUM-accumulated transposes.
  4. BatchNorm: feature-major stats via free-axis reduction + AllReduce of
     per-feature sums, fused scale/shift(+ReLU) activation.

The per-feature bias b is dropped: BN(o + b) == BN(o) exactly.
"""
import os
import sys

sys.path.insert(0, "/opt/trn_rl_repo")

import numpy as np

import concourse.bass as bass
import concourse.bacc as bacc
import concourse.tile as tile
from concourse import mybir
from concourse import bass_utils
from concourse.masks import make_identity

NCORES = 8
P = 128
NEG_SLOPE = 0.2
EPS_BN = 1e-5
SPAD = -60000.0        # pad-row s bias (fp16-safe)

F32 = mybir.dt.float32
F16 = mybir.dt.float16
I32 = mybir.dt.int32


# ----------------------------------------------------------------------------
# host-side graph preprocessing
# ----------------------------------------------------------------------------

def _prep(edge_index, N):
    src = np.asarray(edge_index[0], dtype=np.int64)
    dst = np.asarray(edge_index[1], dtype=np.int64)
    # NOTE: self-loops are handled on-device from the local hs tile, not ELL.

    shard = N // NCORES                      # real nodes per core
    ntiles = (shard + P) // P                # >= 1 pad row per shard
    shard_pad = ntiles * P
    T1 = (ntiles + 1) // 2                   # AG split point (tiles)
    H1 = T1 * P                              # rows in first AG chunk
    H2 = shard_pad - H1

    deg = np.bincount(dst, minlength=N)      # in-degree without self-loop
    order = np.argsort(-deg, kind="stable")  # global degree-descending
    grank = np.empty(N, np.int64)
    grank[order] = np.arange(N)
    core_of = grank % NCORES
    r_of = grank // NCORES                   # local rank on its core

    node_row = core_of * shard_pad + r_of
    pad_row = 0 * shard_pad + shard           # core0's first pad row

    # per (core, tile, partition) edge lists
    c_d, r_d = core_of[dst], r_of[dst]
    t_d, p_d = r_d // P, r_d % P
    okey = (c_d * ntiles + t_d) * P + p_d
    eorder = np.argsort(okey, kind="stable")
    okey_s = okey[eorder]
    rows_s = node_row[src[eorder]]
    boundaries = np.flatnonzero(np.r_[True, okey_s[1:] != okey_s[:-1]])
    run_id = np.zeros(len(okey_s), np.int64)
    run_id[boundaries] = 1
    run_id = np.cumsum(run_id) - 1
    j_in_run = np.arange(len(okey_s)) - boundaries[run_id]

    # slot widths per tile (max over cores & partitions)
    cnt = np.zeros(NCORES * ntiles * P, np.int64)
    np.add.at(cnt, okey, 1)
    S = cnt.reshape(NCORES, ntiles, P).max(axis=(0, 2))
    offs = np.zeros(ntiles + 1, np.int64)
    offs[1:] = np.cumsum(S)
    stot = int(offs[-1])

    idx = np.full((NCORES, P, max(stot, 1)), pad_row, np.int32)
    cc = okey_s // (ntiles * P)
    tt = (okey_s // P) % ntiles
    pp = okey_s % P
    idx[cc, pp, offs[tt] + j_in_run] = rows_s.astype(np.int32)

    # per-core original node id per local rank (for input/output mapping)
    out_nodes = []
    for c in range(NCORES):
        nodes = order[c::NCORES]             # rank order
        out_nodes.append(nodes)

    return {
        "shard": shard, "shard_pad": shard_pad, "ntiles": ntiles,
        "T1": T1, "H1": H1, "H2": H2,
        "S": S.astype(int).tolist(), "offs": offs.astype(int).tolist(),
        "stot": stot, "idx": idx, "node_row": node_row,
        "out_nodes": out_nodes, "pad_row": pad_row,
    }


# ----------------------------------------------------------------------------
# device program
# ----------------------------------------------------------------------------

def _build_program(g, layers, in_dim, ablate=()):
    """layers: list of dicts {H, C, R, hs_off} per layer (R = fp16 elems/row)."""
    ablate = set(ablate)
    shard, shard_pad, ntiles = g["shard"], g["shard_pad"], g["ntiles"]
    T1, H1, H2 = g["T1"], g["H1"], g["H2"]
    S, offs, stot = g["S"], g["offs"], g["stot"]
    nrows = NCORES * shard_pad
    n_l = len(layers)
    Rmax = max(L["R"] for L in layers)
    Hmax = max(L["H"] for L in layers)

    nc = bacc.Bacc("TRN2", target_bir_lowering=False, debug=False, num_devices=NCORES)

    xT = nc.dram_tensor("xT", [in_dim, shard_pad], F16, kind="ExternalInput").ap()
    idx_in = nc.dram_tensor("idx", [P, max(stot, 1)], I32, kind="ExternalInput").ap()
    wexts = [nc.dram_tensor(f"wext{l}", [(in_dim if l == 0 else layers[l - 1]["C"]),
                                         layers[l]["H"] * layers[l]["C"] + 2 * layers[l]["H"]],
                            F16, kind="ExternalInput").ap() for l in range(n_l)]
    gb = nc.dram_tensor("gb", [P, 2 * n_l], F32, kind="ExternalInput").ap()
    spad_in = nc.dram_tensor("spad", [P, 1], F16, kind="ExternalInput").ap()
    out_t = nc.dram_tensor("out", [P, shard_pad], F32, kind="ExternalOutput").ap()

    with tile.TileContext(nc) as tc:
        import contextlib
        with contextlib.ExitStack() as ctx:
            dram = ctx.enter_context(tc.tile_pool(name="dram", bufs=1, space="DRAM"))
            psum = ctx.enter_context(tc.tile_pool(name="psum", bufs=2, space="PSUM"))
            psum_sd = ctx.enter_context(tc.tile_pool(name="psum_sd", bufs=2, space="PSUM"))
            psum_tr = ctx.enter_context(tc.tile_pool(name="psum_tr", bufs=2, space="PSUM"))
            sb = ctx.enter_context(tc.tile_pool(name="sb", bufs=1))
            sb2 = ctx.enter_context(tc.tile_pool(name="sb2", bufs=2))
            sb4 = ctx.enter_context(tc.tile_pool(name="sb4", bufs=4))
            sbe = ctx.enter_context(tc.tile_pool(name="sbe", bufs=3))    # edge small tiles
            hgp = ctx.enter_context(tc.tile_pool(name="hgp", bufs=2))    # gathered rows

            ident = sb.tile([P, P], F16, tag="ident")
            make_identity(nc, ident[:])
            idx_t = sb.tile([P, max(stot, 1)], I32, tag="idx")
            nc.sync.dma_start(idx_t[:], idx_in[:])
            gb_t = sb.tile([P, 2 * n_l], F32, tag="gb")
            nc.sync.dma_start(gb_t[:], gb[:])
            spad_t = sb.tile([P, 1], F16, tag="spad")
            nc.sync.dma_start(spad_t[:], spad_in[:])

            # persistent SBUF state reused across layers
            hs_all = sb.tile([P, ntiles * Rmax], F16, tag="hs_all")
            d_buf = sb.tile([P, ntiles * Hmax], F16, tag="d_buf")
            oT = sb.tile([P, shard_pad], F16, tag="oT")
            yT = sb.tile([P, shard_pad], F16, tag="yT")
            sqscr = sb.tile([P, shard_pad], F16, tag="sqscr")

            for l, L in enumerate(layers):
                H, C, R, s_off = L["H"], L["C"], L["R"], L["hs_off"]
                HC = H * C
                K = in_dim if l == 0 else layers[l - 1]["C"]
                kchunks = K // P

                ag_in = dram.tile([shard_pad, R], F16, tag=f"agin{l}")
                table = dram.tile([nrows, R], F16, tag=f"table{l}", addr_space="Shared")

                wk = []
                for k in range(kchunks):
                    w = sb2.tile([P, HC + 2 * H], F16, tag="wext")
                    nc.sync.dma_start(w[:], wexts[l][k * P:(k + 1) * P, :])
                    wk.append(w)

                # ---- phase 1: local shard matmul -> hs rows + d ----
                for t in range(ntiles):
                    ph = psum.tile([P, HC], F32, tag="mmh", space="PSUM")
                    psd = psum_sd.tile([P, 2 * H], F32, tag="mmsd", space="PSUM")
                    for k in range(kchunks if "mm" not in ablate else 0):
                        if l == 0:
                            lhsT = sb4.tile([P, P], F16, tag="xt")
                            nc.sync.dma_start(lhsT[:], xT[k * P:(k + 1) * P, t * P:(t + 1) * P])
                            lhs_ap = lhsT[:]
                        else:
                            lhs_ap = yT[:, t * P:(t + 1) * P]
                        nc.tensor.matmul(ph[:], lhsT=lhs_ap, rhs=wk[k][:, :HC],
                                         start=(k == 0), stop=(k == kchunks - 1))
                        nc.tensor.matmul(psd[:], lhsT=lhs_ap, rhs=wk[k][:, HC:HC + 2 * H],
                                         start=(k == 0), stop=(k == kchunks - 1))
                    hsv = hs_all[:, t * R:(t + 1) * R]
                    nc.scalar.copy(hsv[:, :HC], ph[:])
                    nc.vector.tensor_copy(hsv[:, s_off:s_off + H], psd[:, :H])
                    if R > s_off + H:
                        nc.vector.memset(hsv[:, s_off + H:], 0.0)
                    nc.vector.tensor_copy(d_buf[:, t * H:(t + 1) * H], psd[:, H:2 * H])
                    if t == ntiles - 1:
                        # pad nodes: s += SPAD so padded slots die in the softmax
                        nc.vector.tensor_tensor(out=hsv[:, s_off:s_off + H],
                                                in0=hsv[:, s_off:s_off + H],
                                                in1=spad_t[:].broadcast_to([P, H]),
                                                op=mybir.AluOpType.add)
                    nc.sync.dma_start(ag_in[t * P:(t + 1) * P, :], hsv)
                    if "coll" not in ablate and t == ntiles - 1:
                        nc.gpsimd.collective_compute(
                            "AllGather", mybir.AluOpType.bypass,
                            replica_groups=[list(range(NCORES))],
                            ins=[ag_in.opt()], outs=[table.opt()],
                        )

                # ---- phase 3: gather + one-pass segment softmax + accumulation ----
                for t in range(ntiles):
                    st = S[t]
                    d_ap = d_buf[:, t * H:(t + 1) * H]
                    hs_t = hs_all[:, t * R:(t + 1) * R]
                    if st > 0:
                        hg = hgp.tile([P, st * R], F16, tag="hg")
                        for j in range(st if "gather" not in ablate else 0):
                            nc.gpsimd.indirect_dma_start(
                                out=hg[:, j * R:(j + 1) * R],
                                out_offset=None,
                                in_=table[:],
                                in_offset=bass.IndirectOffsetOnAxis(
                                    ap=idx_t[:, offs[t] + j:offs[t] + j + 1], axis=0),
                            )
                    if "edge" in ablate:
                        continue
                    # self-loop score
                    es32 = sbe.tile([P, Hmax], F32, tag="es32")
                    nc.vector.tensor_tensor(out=es32[:, :H], in0=hs_t[:, s_off:s_off + H],
                                            in1=d_ap, op=mybir.AluOpType.add)
                    nc.vector.scalar_tensor_tensor(
                        out=es32[:, :H], in0=es32[:, :H], scalar=NEG_SLOPE, in1=es32[:, :H],
                        op0=mybir.AluOpType.mult, op1=mybir.AluOpType.max)
                    mx = sbe.tile([P, Hmax], F32, tag="mx")
                    if st > 0:
                        hg3 = hg[:].rearrange("p (j r) -> p j r", j=st)
                        e32 = sbe.tile([P, Hmax * st], F32, tag="e32")
                        e32v = e32[:, :H * st].rearrange("p (h j) -> p h j", h=H)
                        nc.vector.tensor_tensor(
                            out=e32v,
                            in0=hg3[:, :, s_off:s_off + H].transpose([0, 2, 1]),
                            in1=d_ap.unsqueeze(2).broadcast_to([P, H, st]),
                            op=mybir.AluOpType.add)
                        nc.vector.scalar_tensor_tensor(
                            out=e32v, in0=e32v, scalar=NEG_SLOPE, in1=e32v,
                            op0=mybir.AluOpType.mult, op1=mybir.AluOpType.max)
                        nc.vector.tensor_reduce(out=mx[:, :H], in_=e32v,
                                                axis=mybir.AxisListType.X, op=mybir.AluOpType.max)
                        nc.vector.tensor_tensor(out=mx[:, :H], in0=mx[:, :H], in1=es32[:, :H],
                                                op=mybir.AluOpType.max)
                        # pb = exp(e - m)
                        nc.vector.tensor_tensor(
                            out=e32v, in0=e32v,
                            in1=mx[:, :H].unsqueeze(2).broadcast_to([P, H, st]),
                            op=mybir.AluOpType.subtract)
                        pb16 = sbe.tile([P, Hmax * st], F16, tag="pb16")
                        nc.scalar.activation(pb16[:, :H * st], e32[:, :H * st],
                                             mybir.ActivationFunctionType.Exp)
                        den = sbe.tile([P, Hmax], F32, tag="den")
                        nc.vector.tensor_reduce(
                            out=den[:, :H], in_=pb16[:, :H * st].rearrange("p (h j) -> p h j", h=H),
                            axis=mybir.AxisListType.X, op=mybir.AluOpType.add)
                    else:
                        nc.vector.tensor_copy(mx[:, :H], es32[:, :H])
                        den = sbe.tile([P, Hmax], F32, tag="den")
                        nc.vector.memset(den[:, :H], 0.0)
                    # self term
                    ps32 = sbe.tile([P, Hmax], F32, tag="ps32")
                    nc.vector.tensor_tensor(out=ps32[:, :H], in0=es32[:, :H], in1=mx[:, :H],
                                            op=mybir.AluOpType.subtract)
                    nc.scalar.activation(ps32[:, :H], ps32[:, :H],
                                         mybir.ActivationFunctionType.Exp)
                    nc.vector.tensor_tensor(out=den[:, :H], in0=den[:, :H], in1=ps32[:, :H],
                                            op=mybir.AluOpType.add)
                    # rcp = 1 / (den * H)  (head-mean folded in; den >= 1 always)
                    rcp = sbe.tile([P, Hmax], F32, tag="rcp")
                    if H > 1:
                        nc.vector.tensor_scalar_mul(rcp[:, :H], den[:, :H], float(H))
                        nc.vector.reciprocal(rcp[:, :H], rcp[:, :H])
                    else:
                        nc.vector.reciprocal(rcp[:, :H], den[:, :H])
                    rcp16 = sbe.tile([P, Hmax], F16, tag="rcp16")
                    nc.vector.tensor_copy(rcp16[:, :H], rcp[:, :H])
                    ps16 = sbe.tile([P, Hmax], F16, tag="ps16")
                    nc.vector.tensor_copy(ps16[:, :H], ps32[:, :H])
                    nc.vector.tensor_tensor(out=ps16[:, :H], in0=ps16[:, :H], in1=rcp16[:, :H],
                                            op=mybir.AluOpType.mult)
                    acc = sb2.tile([P, HC], F16, tag="acc")
                    nc.vector.tensor_tensor(
                        out=acc[:].rearrange("p (h c) -> p h c", h=H),
                        in0=hs_t[:, :HC].rearrange("p (h c) -> p h c", h=H),
                        in1=ps16[:, :H].unsqueeze(2).broadcast_to([P, H, C]),
                        op=mybir.AluOpType.mult)
                    if st > 0:
                        # pbn = pb * rcp, then weighted accumulate gathered rows
                        nc.vector.tensor_tensor(
                            out=pb16[:, :H * st].rearrange("p (h j) -> p h j", h=H),
                            in0=pb16[:, :H * st].rearrange("p (h j) -> p h j", h=H),
                            in1=rcp16[:, :H].unsqueeze(2).broadcast_to([P, H, st]),
                            op=mybir.AluOpType.mult)
                        nc.vector.tensor_tensor(
                            out=hg3[:, :, :HC].rearrange("p j (h c) -> p j h c", h=H),
                            in0=hg3[:, :, :HC].rearrange("p j (h c) -> p j h c", h=H),
                            in1=pb16[:, :H * st].rearrange("p (h j) -> p h j", h=H)
                                .transpose([0, 2, 1]).unsqueeze(3).broadcast_to([P, st, H, C]),
                            op=mybir.AluOpType.mult)
                        for j in range(st):
                            nc.vector.tensor_tensor(out=acc[:], in0=acc[:],
                                                    in1=hg3[:, j, :HC],
                                                    op=mybir.AluOpType.add)
                    # head sum on DVE, then one fp16 transpose through PSUM
                    for hh in range(1, H):
                        nc.vector.tensor_tensor(out=acc[:, :C], in0=acc[:, :C],
                                                in1=acc[:, hh * C:(hh + 1) * C],
                                                op=mybir.AluOpType.add)
                    ptr = psum_tr.tile([P, P], F16, tag="tr", space="PSUM")
                    nc.tensor.matmul(ptr[:], lhsT=acc[:, :C], rhs=ident[:],
                                     is_transpose=True, start=True, stop=True)
                    nc.scalar.copy(oT[:, t * P:(t + 1) * P], ptr[:])

                # ---- phase 4: batchnorm (+relu) ----
                nsum = sb4.tile([P, 1], F32, tag="nsum")
                nsq = sb4.tile([P, 1], F32, tag="nsq")
                nc.vector.tensor_reduce(out=nsum[:], in_=oT[:], axis=mybir.AxisListType.X,
                                        op=mybir.AluOpType.add)
                nc.scalar.activation(sqscr[:], oT[:], mybir.ActivationFunctionType.Square,
                                     accum_out=nsq[:])
                ar_in = dram.tile([P, 2], F32, tag=f"arin{l}")
                ar_out = dram.tile([P, 2], F32, tag=f"arout{l}", addr_space="Shared")
                st2 = sb4.tile([P, 2], F32, tag="st2")
                nc.vector.tensor_copy(st2[:, 0:1], nsum[:])
                nc.vector.tensor_copy(st2[:, 1:2], nsq[:])
                nc.sync.dma_start(ar_in[:], st2[:])
                if "coll" not in ablate:
                    nc.gpsimd.collective_compute(
                        "AllReduce", mybir.AluOpType.add,
                        replica_groups=[list(range(NCORES))],
                        ins=[ar_in.opt()], outs=[ar_out.opt()],
                    )
                stg = sb4.tile([P, 2], F32, tag="stg")
                nc.sync.dma_start(stg[:], ar_out[:])
                ntotal = float(NCORES * shard)
                mu = sb4.tile([P, 1], F32, tag="mu")
                nc.vector.tensor_scalar_mul(mu[:], stg[:, 0:1], 1.0 / ntotal)
                var = sb4.tile([P, 1], F32, tag="var")
                nc.vector.tensor_scalar_mul(var[:], stg[:, 1:2], 1.0 / ntotal)
                musq = sb4.tile([P, 1], F32, tag="musq")
                nc.vector.tensor_tensor(out=musq[:], in0=mu[:], in1=mu[:], op=mybir.AluOpType.mult)
                nc.vector.tensor_tensor(out=var[:], in0=var[:], in1=musq[:], op=mybir.AluOpType.subtract)
                rstd = sb4.tile([P, 1], F32, tag="rstd")
                nc.vector.tensor_scalar_add(var[:], var[:], EPS_BN)
                nc.scalar.activation(rstd[:], var[:], mybir.ActivationFunctionType.Sqrt)
                nc.vector.reciprocal(rstd[:], rstd[:])
                scale = sb4.tile([P, 1], F32, tag="scale")
                nc.vector.tensor_tensor(out=scale[:], in0=gb_t[:, 2 * l:2 * l + 1], in1=rstd[:],
                                        op=mybir.AluOpType.mult)
                shift = sb4.tile([P, 1], F32, tag="shift")
                nc.vector.tensor_tensor(out=shift[:], in0=mu[:], in1=scale[:], op=mybir.AluOpType.mult)
                nc.vector.tensor_tensor(out=shift[:], in0=gb_t[:, 2 * l + 1:2 * l + 2], in1=shift[:],
                                        op=mybir.AluOpType.subtract)
                if l < n_l - 1:
                    nc.scalar.activation(yT[:], oT[:], mybir.ActivationFunctionType.Relu,
                                         bias=shift[:], scale=scale[:])
                    npad = shard_pad - shard
                    if npad > 0:
                        nc.vector.memset(yT[:, shard:], 0.0)
                else:
                    outf = sb.tile([P, shard_pad], F32, tag="outf")
                    nc.scalar.activation(outf[:], oT[:], mybir.ActivationFunctionType.Identity,
                                         bias=shift[:], scale=scale[:])
                    nc.sync.dma_start(out_t[:], outf[:])

    nc.compile()
    return nc


# ----------------------------------------------------------------------------
# entry point
# ----------------------------------------------------------------------------

def _make_layers(params):
    layers = []
    for (W, asr, ads, gmm, bet) in params:
        H, C = asr.shape
        HC = H * C
        R = ((HC + H) * 2 + 15) // 16 * 8      # fp16 elems per row, 16B-aligned
        layers.append({"H": H, "C": C, "R": R, "hs_off": HC})
    return layers


def _make_inputs(x, g, params, layers, in_dim):
    wexts = []
    for (W, asr, ads, gmm, bet), L in zip(params, layers):
        H, C = L["H"], L["C"]
        w_s = np.einsum("khc,hc->kh", W.reshape(W.shape[0], H, C), asr)
        w_d = np.einsum("khc,hc->kh", W.reshape(W.shape[0], H, C), ads)
        wexts.append(np.concatenate([W, w_s, w_d], axis=1).astype(np.float16))
    n_l = len(layers)
    gbm = np.zeros((P, 2 * n_l), np.float32)
    for l, (W, asr, ads, gmm, bet) in enumerate(params):
        gbm[:len(gmm), 2 * l] = gmm
        gbm[:len(bet), 2 * l + 1] = bet
    shard, shard_pad = g["shard"], g["shard_pad"]
    spad = np.zeros((P, 1), np.float16)
    lastbase = (g["ntiles"] - 1) * P
    for p in range(P):
        if lastbase + p >= shard:
            spad[p, 0] = SPAD
    in_maps = []
    for c in range(NCORES):
        nodes = g["out_nodes"][c]
        xT_c = np.zeros((in_dim, shard_pad), np.float16)
        xT_c[:, :shard] = x[nodes].T.astype(np.float16)
        m = {"xT": xT_c, "idx": np.ascontiguousarray(g["idx"][c]), "gb": gbm, "spad": spad}
        for l, w in enumerate(wexts):
            m[f"wext{l}"] = w
        in_maps.append(m)
    return in_maps


def build_for_inputs(x, edge_index, params_list, ablate=(), nlayers=3):
    x = np.asarray(x, np.float32)
    N, in_dim = x.shape
    g = _prep(np.asarray(edge_index), N)
    params = params_list[:nlayers]
    layers = _make_layers(params)
    nc = _build_program(g, layers, in_dim, ablate=ablate)
    in_maps = _make_inputs(x, g, params, layers, in_dim)
    return nc, in_maps, g, layers


def kernel(x, edge_index,
           W0, a_src0, a_dst0, b0, gamma0, beta0,
           W1, a_src1, a_dst1, b1, gamma1, beta1,
           W2, a_src2, a_dst2, b2, gamma2, beta2, _profile=None, _nlayers=3):
    x = np.asarray(x, np.float32)
    N, in_dim = x.shape

    params = [(np.asarray(W0, np.float32), np.asarray(a_src0, np.float32), np.asarray(a_dst0, np.float32),
               np.asarray(gamma0, np.float32), np.asarray(beta0, np.float32)),
              (np.asarray(W1, np.float32), np.asarray(a_src1, np.float32), np.asarray(a_dst1, np.float32),
               np.asarray(gamma1, np.float32), np.asarray(beta1, np.float32)),
              (np.asarray(W2, np.float32), np.asarray(a_src2, np.float32), np.asarray(a_dst2, np.float32),
               np.asarray(gamma2, np.float32), np.asarray(beta2, np.float32))][:_nlayers]

    nc, in_maps, g, layers = build_for_inputs(x, np.asarray(edge_index), params, nlayers=_nlayers)

    if _profile is not None:
        _profile["nc"] = nc
        _profile["in_maps"] = in_maps
    res = bass_utils.run_bass_kernel_spmd(nc, in_maps, core_ids=list(range(NCORES)))

    C_out = layers[-1]["C"]
    out = np.empty((N, C_out), np.float32)
    for c in range(NCORES):
        yT = res.results[c]["out"]           # [P(feat), shard_pad]
        out[g["out_nodes"][c]] = yT[:C_out, :g["shard"]].T
    if _profile is not None:
        _profile["results"] = res
    return out
